# revision 23
# baseline (speedup 1.0000x reference)
"""CpxRBM translation-invariant log-psi kernel for 8 Trainium2 NeuronCores.

Computes sum(log(cosh(sym @ W.T))) where sym is the (4095, 4096) matrix of
circular shifts of v = 2*vis_states - 1 and W is (1024, 4096) complex64.

Strategy (shift-sharded, 512 shifts/core; core 7 computes the extra wrap
shift s=4095 as real data and the host subtracts its exact contribution):
  - fp8 e4m3 DoubleRow matmuls (2x bf16 throughput).  Weights are scaled by
    S=2048 and quantized to e4m3 (rel err ~2.5e-3 on the final sum, vs 2e-2
    tolerance); sym values are +-1, exact in fp8.  The host pre-builds the
    full DoubleRow-layout sym tensor so it lands in 2 contiguous DMAs.
  - Orientation: weights stationary [128k, 2j, 128o], sym moving
    [128k, 2j, 512s], psum out [128 o-partitions, 512 shifts].  16 k-double-
    chunks accumulate per (o-block, re/im); 8 o-blocks x 2 = 256 matmuls.
  - log(cosh(x+iy)) elementwise, o-blocks in pairs on [128, 2, 512] tiles:
      t1 = 2cosh x = e^x + e^-x;  q = |2cosh z|^2 = t1^2 - 4 sin^2 y
      Re-part: 0.5*ln(q) - ln2 (Ln accumulated per partition)
      Im-part: 2*atan(b/(r+a)), a = t1*cos y, b = (e^x-e^-x)*sin y,
               r = sqrt(q) = exp(0.5 ln q)   (exact principal atan2)
    Sin is table-accurate to |arg|<~3.3 and sigma_y ~ 0.64, so psum feeds
    Sin directly (no range reduction); cos y = Sin(y + pi/2) (the y > 1.7
    tail only perturbs the tiny Im part); sin^2 via ACT Square;
    1/(r+a) via reciprocal_approx_fast (DVE).
  - ACT table sets: trig_and_small {Sin, Arctan}, natural_log_exp_and_others
    {Exp, Ln}; the Arctan of pair k flushes during pair k+1's trig phase ->
    2 table loads per pair.
  - Per-core output: (128, 8) fp32 partial sums; host reduces.
"""
import math
import numpy as np
import ml_dtypes
from contextlib import ExitStack

import concourse.bass as bass
import concourse.mybir as mybir
import concourse.tile as tile
from concourse import bacc
from concourse.bass_utils import run_bass_kernel_spmd
from concourse.hw_specs import get_activation_tables
import bass_rust as _bass_rust

F32 = mybir.dt.float32
FP8 = mybir.dt.float8e4
AF = mybir.ActivationFunctionType
ALU = mybir.AluOpType
DR = mybir.MatmulPerfMode.DoubleRow

PI = float(np.pi)
VIS_N = 4096
INP_N = 4096
OUP_N = 1024
N_CORES = 8
S_PER_CORE = 512
WIN = S_PER_CORE + INP_N  # 4608
N_C2 = 16                 # k double-chunks (2x128 each)
N_OB = 8                  # o-blocks of 128
N_PAIR = N_OB // 2
N_GROUP = 5               # elementwise groups: (0,1),(2,3),(4,5),(6),(7)
SCALE = 2048.0
INV_S = 1.0 / SCALE

_ALLOWED_SETS = {"natural_log_exp_and_others", "trig_and_small"}


class _Bacc(bacc.Bacc):
    def insert_act_table_loads(self):
        has_activation = any(
            isinstance(i, mybir.InstActivation)
            for b in self.main_func.blocks
            for i in b.instructions
        )
        if not has_activation:
            return
        tables = [
            (name, funcs if name in _ALLOWED_SETS else set())
            for name, funcs in get_activation_tables(self.m.arch).items()
        ]
        _bass_rust.insert_act_table_loads(self, tables)


_nc_cache = None
last_results = None


def _build_nc():
    nc = _Bacc("TRN2", target_bir_lowering=False, debug=False)

    # symd[p, c2, j, s] = +-1 window value at vwin[256*c2 + 128*j + p + s]
    symd = nc.dram_tensor("symd", [128, N_C2, 2, S_PER_CORE], FP8, kind="ExternalInput")
    # wt[ob, reim, p, (c2, j, o)]; per (ob, reim) one contiguous 512KB slab
    wt = nc.dram_tensor("wt", [N_OB, 2, 128, N_C2, 2, 128], FP8, kind="ExternalInput")
    acc = nc.dram_tensor("acc", [128, 2 * N_GROUP], F32, kind="ExternalOutput")

    with tile.TileContext(nc) as tc, ExitStack() as ctx:
        singles = ctx.enter_context(tc.tile_pool(name="singles", bufs=1))
        ppool = ctx.enter_context(tc.tile_pool(name="ppool", bufs=2, space="PSUM"))
        stage = ctx.enter_context(tc.tile_pool(name="stage", bufs=1))

        acc_sb = singles.tile([128, 2 * N_GROUP], F32)
        half_pi = singles.tile([128, 1], F32)
        nc.vector.memset(half_pi, PI / 2.0)
        ln_eps = singles.tile([128, 1], F32)
        nc.vector.memset(ln_eps, 1e-5)

        # sym in 4 contiguous DMAs (small first chunk) so matmuls start early
        symall = singles.tile([128, N_C2, 2, S_PER_CORE], FP8, name="symall")
        for lo, hi in ((0, 4), (4, 10), (10, 16)):
            nc.sync.dma_start(out=symall[:, lo:hi], in_=symd[:, lo:hi])
        # weight slabs (contiguous per partition) on the idle gpsimd queue;
        # ob0's slabs are split so the first matmul's stationary lands first
        wtiles = {}
        for ob in range(N_OB):
            for r in range(2):
                w_t = singles.tile(
                    [128, N_C2, 2, 128], FP8, tag=f"w{ob % 4}_{r}", name=f"w{ob}_{r}"
                )
                if ob == 0:
                    nc.gpsimd.dma_start(out=w_t[:, 0:2], in_=wt[ob, r, :, 0:2])
                    nc.gpsimd.dma_start(out=w_t[:, 2:16], in_=wt[ob, r, :, 2:16])
                else:
                    nc.gpsimd.dma_start(out=w_t, in_=wt[ob, r])
                wtiles[(ob, r)] = w_t

        # ACT-table phase ordering: each phase's ops depend on the previous
        # phase's so the scheduler cannot interleave table sets.  The Ln/sqrt
        # of group g are LAGGED into group g+1's exp phase (their input sits
        # behind group g's DVE chain; un-lagged they serialize the pipeline),
        # and arctans are deferred two groups.  The last two groups (singles,
        # after the final matmul) run un-lagged to shorten the drain.
        state = {"prev_phase": [], "at_pend": [], "lnr_pend": [], "gtiles": {}}

        def phase(ops):
            prev = state["prev_phase"]
            for a in ops:
                for b in prev:
                    tile.add_dep_helper(a.ins, b.ins, reason="act-set order")
            state["prev_phase"] = ops

        def flush_at(gmax):
            ops = []
            while state["at_pend"] and state["at_pend"][0][1] <= gmax:
                bv, _pg, col = state["at_pend"].pop(0)
                sc = stage.tile(list(bv.shape), F32, tag=f"at_scratch{bv.shape[1]}", bufs=2)
                ops.append(
                    nc.scalar.activation(
                        sc, bv, AF.Arctan,
                        accum_out=acc_sb[:, col : col + 1],
                    )
                )
            return ops

        def emit_lnr(pg):
            """Ln(q)+accum and sqrt for group pg (exp-table ops)."""
            t = state["gtiles"][pg]
            i_ln = nc.scalar.activation(
                t["lnq"], t["qq"], AF.Ln, bias=ln_eps,
                accum_out=acc_sb[:, 2 * pg : 2 * pg + 1],
            )
            i_r = nc.scalar.activation(t["rr"], t["lnq"], AF.Exp, scale=0.5)
            return [i_ln, i_r]

        def emit_avchain(pg):
            """Im-path DVE chain for group pg (needs rr)."""
            t = state["gtiles"][pg]
            av, bv, rec = t["av"], t["bv"], t["rec"]
            nc.vector.tensor_tensor(av, t["t1"], t["cy"], ALU.mult)   # a
            nc.vector.tensor_tensor(bv, t["t2"], t["sy"], ALU.mult)   # b
            nc.vector.tensor_tensor(av, t["rr"], av, ALU.add)         # den = r+a
            nc.vector.reciprocal_approx_fast(rec, av)
            nc.vector.tensor_tensor(bv, bv, rec, ALU.mult)            # t = b/den
            state["at_pend"].append((bv, pg, 2 * pg + 1))

        def emit_group(g, ps_r, ps_i, width, lagged):
            """Elementwise log-cosh for group g on [128, width, 512] psum."""
            shp = [128, width, S_PER_CORE]
            nbufs = {"bv": 3, "sy": 2, "cy": 2, "t1": 2, "t2": 2, "qq": 2, "rr": 2,
                     "sq": 1, "ep": 1, "em": 1, "lnq": 1, "av": 1, "rec": 1}
            t = {
                n: stage.tile(shp, F32, tag=f"{n}{width}", name=f"{n}{width}_{g}",
                              bufs=nbufs[n])
                for n in nbufs
            }
            state["gtiles"][g] = t

            # --- trig phase: sy, cy (per source psum tile) + old arctans ---
            ops = []
            for i, ps in enumerate(ps_i):
                ops.append(
                    nc.scalar.activation(t["sy"][:, i : i + 1, :], ps, AF.Sin, scale=INV_S)
                )
            for i, ps in enumerate(ps_i):
                ops.append(
                    nc.scalar.activation(
                        t["cy"][:, i : i + 1, :], ps, AF.Sin, scale=INV_S, bias=half_pi
                    )
                )
            phase(ops + flush_at(gmax=g - 2))
            nc.vector.tensor_tensor(t["sq"], t["sy"], t["sy"], ALU.mult)

            # --- exp phase: lagged Ln/sqrt of prior groups + this group's exps
            ops = []
            while state["lnr_pend"]:
                pg = state["lnr_pend"].pop(0)
                ops += emit_lnr(pg)
                # Im-path chain right away: r(pg) is its only fresh input, and
                # it must clear the DVE queue before this group's q-chain.
                emit_avchain(pg)
                state["gtiles"][pg]["done"] = True
            for i, ps in enumerate(ps_r):
                ops.append(
                    nc.scalar.activation(t["ep"][:, i : i + 1, :], ps, AF.Exp, scale=INV_S)
                )
            for i, ps in enumerate(ps_r):
                ops.append(
                    nc.scalar.activation(t["em"][:, i : i + 1, :], ps, AF.Exp, scale=-INV_S)
                )
            # DVE q-chain (q = t1^2 - 4 sin^2 y; Ln bias keeps it positive)
            nc.vector.tensor_tensor(t["t1"], t["ep"], t["em"], ALU.add)
            nc.vector.tensor_tensor(t["t2"], t["ep"], t["em"], ALU.subtract)
            nc.vector.tensor_tensor(t["qq"], t["t1"], t["t1"], ALU.mult)
            nc.vector.scalar_tensor_tensor(t["qq"], t["sq"], -4.0, t["qq"], ALU.mult, ALU.add)
            if lagged:
                phase(ops)
                state["lnr_pend"].append(g)
            else:
                ops += emit_lnr(g)
                phase(ops)
                emit_avchain(g)
                state["gtiles"][g]["done"] = True

        # per-o-block one-bank psum tiles (fine-grained WAR release, so late
        # matmuls never wait on a pair partner's elementwise); elementwise
        # groups are the first three pairs plus the last two blocks singly.
        g = 0
        grp_r, grp_i = [], []
        for ob in range(N_OB):
            ps_r = ppool.tile([128, 1, S_PER_CORE], F32, tag=f"psr{ob % 2}",
                              name=f"psr{ob}")
            ps_i = ppool.tile([128, 1, S_PER_CORE], F32, tag=f"psi{ob % 2}",
                              name=f"psi{ob}")
            for r, ps in ((0, ps_r), (1, ps_i)):
                w_t = wtiles[(ob, r)]
                for c2 in range(N_C2):
                    nc.tensor.matmul(
                        ps[:, 0, :],
                        w_t[:, c2, :, :],
                        symall[:, c2, :, :],
                        start=(c2 == 0),
                        stop=(c2 == N_C2 - 1),
                        perf_mode=DR,
                    )
            grp_r.append(ps_r)
            grp_i.append(ps_i)
            if (ob < 6 and ob % 2 == 1) or ob >= 6:
                emit_group(g, grp_r, grp_i, width=len(grp_r),
                           lagged=(ob < 4))
                g += 1
                grp_r, grp_i = [], []

        phase(flush_at(gmax=99))

        nc.sync.dma_start(out=acc[:, :], in_=acc_sb)

    nc.finalize()
    return nc


def _get_nc():
    global _nc_cache
    if _nc_cache is None:
        _nc_cache = _build_nc()
    return _nc_cache


_sym_idx_cache = None


def _sym_idx():
    global _sym_idx_cache
    if _sym_idx_cache is None:
        p = np.arange(128)[:, None, None, None]
        c2 = np.arange(N_C2)[None, :, None, None]
        j = np.arange(2)[None, None, :, None]
        s = np.arange(S_PER_CORE)[None, None, None, :]
        _sym_idx_cache = (256 * c2 + 128 * j + p + s).astype(np.int64)
    return _sym_idx_cache


def kernel(vis_states: np.ndarray, weights: np.ndarray) -> np.ndarray:
    global last_results
    vis = np.asarray(vis_states).astype(np.float32)
    v = 2.0 * vis - 1.0                       # {-1, +1}
    vv = np.concatenate([v, v]).astype(ml_dtypes.float8_e4m3)  # exact in fp8
    w = np.asarray(weights)

    # quantize scaled weights to e4m3 (TRN FP8_EXP4 max +-240)
    wr = np.clip(w.real.astype(np.float64) * SCALE, -240, 240)
    wi = np.clip(w.imag.astype(np.float64) * SCALE, -240, 240)
    wr8 = wr.astype(ml_dtypes.float8_e4m3)
    wi8 = wi.astype(ml_dtypes.float8_e4m3)

    # wt[ob, r, p, c2, j, o] = W8[r][ob*128+o, (2*c2+j)*128+p]
    wt = np.empty((N_OB, 2, 128, N_C2, 2, 128), dtype=ml_dtypes.float8_e4m3)
    for r, w8 in ((0, wr8), (1, wi8)):
        a = w8.T.reshape(N_C2, 2, 128, N_OB, 128)   # [c2, j, p, ob, o]
        wt[:, r] = a.transpose(3, 2, 0, 1, 4)        # [ob, p, c2, j, o]

    idx = _sym_idx()
    in_maps = []
    for c in range(N_CORES):
        win = vv[c * S_PER_CORE : c * S_PER_CORE + WIN]
        in_maps.append({"symd": np.ascontiguousarray(win[idx]), "wt": wt})

    nc = _get_nc()
    res = run_bass_kernel_spmd(nc, in_maps, core_ids=list(range(N_CORES)))
    last_results = res

    tot_ln = 0.0
    tot_at = 0.0
    for r in res.results:
        a = r["acc"].astype(np.float64)
        tot_ln += a[:, 0::2].sum()
        tot_at += a[:, 1::2].sum()

    n_counted = N_CORES * S_PER_CORE * OUP_N  # includes the wrap shift s=4095
    real = 0.5 * tot_ln - math.log(2.0) * n_counted
    imag = 2.0 * tot_at

    # subtract the wrap shift's exact contribution (same quantized weights)
    w_eff = (wr8.astype(np.float64) + 1j * wi8.astype(np.float64)) / SCALE
    v4095 = v.astype(np.float64)[(4095 + np.arange(INP_N)) % VIS_N]
    pre = w_eff @ v4095
    f4095 = np.sum(np.log(np.cosh(pre)))
    real -= f4095.real
    imag -= f4095.imag
    return np.array(real + 1j * imag, dtype=np.complex64)


# revision 25
# speedup vs baseline: 1.0939x; 1.0939x over previous
"""CpxRBM translation-invariant log-psi kernel for 8 Trainium2 NeuronCores.

Computes sum(log(cosh(sym @ W.T))) where sym is the (4095, 4096) matrix of
circular shifts of v = 2*vis_states - 1 and W is (1024, 4096) complex64.

Strategy (shift-sharded, 512 shifts/core; core 7 computes the extra wrap
shift s=4095 as real data and the host subtracts its exact contribution):
  - fp8 e4m3 DoubleRow matmuls (2x bf16 throughput).  Weights are scaled by
    S=2048 and quantized to e4m3 (rel err ~2.5e-3 on the final sum, vs 2e-2
    tolerance); sym values are +-1, exact in fp8.  The host pre-builds the
    full DoubleRow-layout sym tensor so it lands in 2 contiguous DMAs.
  - Orientation: weights stationary [128k, 2j, 128o], sym moving
    [128k, 2j, 512s], psum out [128 o-partitions, 512 shifts].  16 k-double-
    chunks accumulate per (o-block, re/im); 8 o-blocks x 2 = 256 matmuls.
  - log(cosh(x+iy)) elementwise, o-blocks in pairs on [128, 2, 512] tiles:
      t1 = 2cosh x = e^x + e^-x;  q = |2cosh z|^2 = t1^2 - 4 sin^2 y
      Re-part: 0.5*ln(q) - ln2 (Ln accumulated per partition)
      Im-part: 2*atan(b/(r+a)), a = t1*cos y, b = (e^x-e^-x)*sin y,
               r = sqrt(q) = exp(0.5 ln q)   (exact principal atan2)
    Sin is table-accurate to |arg|<~3.3 and sigma_y ~ 0.64, so psum feeds
    Sin directly (no range reduction); cos y = Sin(y + pi/2) (the y > 1.7
    tail only perturbs the tiny Im part); sin^2 via ACT Square;
    1/(r+a) via reciprocal_approx_fast (DVE).
  - ACT table sets: trig_and_small {Sin, Arctan}, natural_log_exp_and_others
    {Exp, Ln}; the Arctan of pair k flushes during pair k+1's trig phase ->
    2 table loads per pair.
  - Per-core output: (128, 8) fp32 partial sums; host reduces.
"""
import math
import numpy as np
import ml_dtypes
from contextlib import ExitStack

import concourse.bass as bass
import concourse.mybir as mybir
import concourse.tile as tile
from concourse import bacc
from concourse.bass_utils import run_bass_kernel_spmd
from concourse.hw_specs import get_activation_tables
import bass_rust as _bass_rust

F32 = mybir.dt.float32
FP8 = mybir.dt.float8e4
AF = mybir.ActivationFunctionType
ALU = mybir.AluOpType
DR = mybir.MatmulPerfMode.DoubleRow

PI = float(np.pi)
VIS_N = 4096
INP_N = 4096
OUP_N = 1024
N_CORES = 8
S_PER_CORE = 512
WIN = S_PER_CORE + INP_N  # 4608
N_C2 = 16                 # k double-chunks (2x128 each)
N_OB = 8                  # o-blocks of 128
N_PAIR = N_OB // 2
N_GROUP = 5               # elementwise groups: (0,1),(2,3),(4,5),(6),(7)
SCALE = 2048.0
INV_S = 1.0 / SCALE

_ALLOWED_SETS = {"natural_log_exp_and_others", "trig_and_small"}


class _Bacc(bacc.Bacc):
    def insert_act_table_loads(self):
        has_activation = any(
            isinstance(i, mybir.InstActivation)
            for b in self.main_func.blocks
            for i in b.instructions
        )
        if not has_activation:
            return
        tables = [
            (name, funcs if name in _ALLOWED_SETS else set())
            for name, funcs in get_activation_tables(self.m.arch).items()
        ]
        _bass_rust.insert_act_table_loads(self, tables)


_nc_cache = None
last_results = None


def _build_nc():
    nc = _Bacc("TRN2", target_bir_lowering=False, debug=False)

    # symd[p, c2, j, s] = +-1 window value at vwin[256*c2 + 128*j + p + s]
    symd = nc.dram_tensor("symd", [128, N_C2, 2, S_PER_CORE], FP8, kind="ExternalInput")
    # wt[ob, reim, p, (c2, j, o)]; per (ob, reim) one contiguous 512KB slab
    wt = nc.dram_tensor("wt", [N_OB, 2, 128, N_C2, 2, 128], FP8, kind="ExternalInput")
    acc = nc.dram_tensor("acc", [128, 2 * N_GROUP], F32, kind="ExternalOutput")

    with tile.TileContext(nc) as tc, ExitStack() as ctx:
        singles = ctx.enter_context(tc.tile_pool(name="singles", bufs=1))
        ppool = ctx.enter_context(tc.tile_pool(name="ppool", bufs=2, space="PSUM"))
        stage = ctx.enter_context(tc.tile_pool(name="stage", bufs=1))

        acc_sb = singles.tile([128, 2 * N_GROUP], F32)
        half_pi = singles.tile([128, 1], F32)
        nc.vector.memset(half_pi, PI / 2.0)
        ln_eps = singles.tile([128, 1], F32)
        nc.vector.memset(ln_eps, 1e-5)

        # sym in 4 contiguous DMAs, triggers spread across idle queues so
        # chunks land in parallel just ahead of the matmul stream
        symall = singles.tile([128, N_C2, 2, S_PER_CORE], FP8, name="symall")
        for eng, (lo, hi) in zip(
            (nc.sync, nc.scalar, nc.scalar, nc.sync),
            ((0, 2), (2, 6), (6, 11), (11, 16)),
        ):
            eng.dma_start(out=symall[:, lo:hi], in_=symd[:, lo:hi])
        # weight slabs (contiguous per partition) on the idle gpsimd queue;
        # ob0's slabs are split so the first matmul's stationary lands first
        wtiles = {}
        for ob in range(N_OB):
            for r in range(2):
                w_t = singles.tile(
                    [128, N_C2, 2, 128], FP8, tag=f"w{ob % 4}_{r}", name=f"w{ob}_{r}"
                )
                if ob == 0:
                    nc.gpsimd.dma_start(out=w_t[:, 0:2], in_=wt[ob, r, :, 0:2])
                    nc.gpsimd.dma_start(out=w_t[:, 2:16], in_=wt[ob, r, :, 2:16])
                else:
                    nc.gpsimd.dma_start(out=w_t, in_=wt[ob, r])
                wtiles[(ob, r)] = w_t

        # ACT-table phase ordering: each phase's ops depend on the previous
        # phase's so the scheduler cannot interleave table sets.  The Ln/sqrt
        # of group g are LAGGED into group g+1's exp phase (their input sits
        # behind group g's DVE chain; un-lagged they serialize the pipeline),
        # and arctans are deferred two groups.  The last two groups (singles,
        # after the final matmul) run un-lagged to shorten the drain.
        state = {"prev_phase": [], "at_pend": [], "lnr_pend": [], "gtiles": {}}

        def phase(ops):
            prev = state["prev_phase"]
            for a in ops:
                for b in prev:
                    tile.add_dep_helper(a.ins, b.ins, reason="act-set order")
            state["prev_phase"] = ops

        def flush_at(gmax):
            ops = []
            while state["at_pend"] and state["at_pend"][0][1] <= gmax:
                bv, _pg, col = state["at_pend"].pop(0)
                sc = stage.tile(list(bv.shape), F32, tag=f"at_scratch{bv.shape[1]}", bufs=2)
                ops.append(
                    nc.scalar.activation(
                        sc, bv, AF.Arctan,
                        accum_out=acc_sb[:, col : col + 1],
                    )
                )
            return ops

        def emit_lnr(pg):
            """Ln(q)+accum and sqrt for group pg (exp-table ops)."""
            t = state["gtiles"][pg]
            i_ln = nc.scalar.activation(
                t["lnq"], t["qq"], AF.Ln, bias=ln_eps,
                accum_out=acc_sb[:, 2 * pg : 2 * pg + 1],
            )
            i_r = nc.scalar.activation(t["rr"], t["lnq"], AF.Exp, scale=0.5)
            return [i_ln, i_r]

        def emit_avchain(pg):
            """Im-path DVE chain for group pg (needs rr)."""
            t = state["gtiles"][pg]
            av, bv, rec = t["av"], t["bv"], t["rec"]
            nc.vector.tensor_tensor(av, t["t1"], t["cy"], ALU.mult)   # a
            nc.vector.tensor_tensor(bv, t["t2"], t["sy"], ALU.mult)   # b
            nc.vector.tensor_tensor(av, t["rr"], av, ALU.add)         # den = r+a
            nc.vector.reciprocal_approx_fast(rec, av)
            nc.vector.tensor_tensor(bv, bv, rec, ALU.mult)            # t = b/den
            state["at_pend"].append((bv, pg, 2 * pg + 1))

        def emit_group(g, ps_r, ps_i, width, lagged):
            """Elementwise log-cosh for group g on [128, width, 512] psum."""
            shp = [128, width, S_PER_CORE]
            nbufs = {"bv": 3, "sy": 2, "cy": 2, "t1": 2, "t2": 2, "qq": 2, "rr": 2,
                     "sq": 1, "ep": 1, "em": 1, "lnq": 1, "av": 1, "rec": 1}
            t = {
                n: stage.tile(shp, F32, tag=f"{n}{width}", name=f"{n}{width}_{g}",
                              bufs=nbufs[n])
                for n in nbufs
            }
            state["gtiles"][g] = t

            # --- trig phase: sy, cy (per source psum tile) + old arctans ---
            ops = []
            for i, ps in enumerate(ps_i):
                ops.append(
                    nc.scalar.activation(t["sy"][:, i : i + 1, :], ps, AF.Sin, scale=INV_S)
                )
            for i, ps in enumerate(ps_i):
                ops.append(
                    nc.scalar.activation(
                        t["cy"][:, i : i + 1, :], ps, AF.Sin, scale=INV_S, bias=half_pi
                    )
                )
            phase(ops + flush_at(gmax=g - 2))
            nc.vector.tensor_tensor(t["sq"], t["sy"], t["sy"], ALU.mult)

            # --- exp phase: lagged Ln/sqrt of prior groups + this group's exps
            ops = []
            while state["lnr_pend"]:
                pg = state["lnr_pend"].pop(0)
                ops += emit_lnr(pg)
                # Im-path chain right away: r(pg) is its only fresh input, and
                # it must clear the DVE queue before this group's q-chain.
                emit_avchain(pg)
                state["gtiles"][pg]["done"] = True
            for i, ps in enumerate(ps_r):
                ops.append(
                    nc.scalar.activation(t["ep"][:, i : i + 1, :], ps, AF.Exp, scale=INV_S)
                )
            for i, ps in enumerate(ps_r):
                ops.append(
                    nc.scalar.activation(t["em"][:, i : i + 1, :], ps, AF.Exp, scale=-INV_S)
                )
            # DVE q-chain (q = t1^2 - 4 sin^2 y; Ln bias keeps it positive)
            nc.vector.tensor_tensor(t["t1"], t["ep"], t["em"], ALU.add)
            nc.vector.tensor_tensor(t["t2"], t["ep"], t["em"], ALU.subtract)
            nc.vector.tensor_tensor(t["qq"], t["t1"], t["t1"], ALU.mult)
            nc.vector.scalar_tensor_tensor(t["qq"], t["sq"], -4.0, t["qq"], ALU.mult, ALU.add)
            if lagged:
                phase(ops)
                state["lnr_pend"].append(g)
            else:
                ops += emit_lnr(g)
                phase(ops)
                emit_avchain(g)
                state["gtiles"][g]["done"] = True

        # per-o-block one-bank psum tiles (fine-grained WAR release, so late
        # matmuls never wait on a pair partner's elementwise); elementwise
        # groups are the first three pairs plus the last two blocks singly.
        g = 0
        grp_r, grp_i = [], []
        for ob in range(N_OB):
            ps_r = ppool.tile([128, 1, S_PER_CORE], F32, tag=f"psr{ob % 2}",
                              name=f"psr{ob}")
            ps_i = ppool.tile([128, 1, S_PER_CORE], F32, tag=f"psi{ob % 2}",
                              name=f"psi{ob}")
            for r, ps in ((0, ps_r), (1, ps_i)):
                w_t = wtiles[(ob, r)]
                for c2 in range(N_C2):
                    nc.tensor.matmul(
                        ps[:, 0, :],
                        w_t[:, c2, :, :],
                        symall[:, c2, :, :],
                        start=(c2 == 0),
                        stop=(c2 == N_C2 - 1),
                        perf_mode=DR,
                    )
            grp_r.append(ps_r)
            grp_i.append(ps_i)
            if (ob < 6 and ob % 2 == 1) or ob >= 6:
                emit_group(g, grp_r, grp_i, width=len(grp_r), lagged=True)
                g += 1
                grp_r, grp_i = [], []

        # drain: final Ln/sqrt exp phase, its Im chain, then all arctans
        ops = []
        while state["lnr_pend"]:
            pg = state["lnr_pend"].pop(0)
            ops += emit_lnr(pg)
            emit_avchain(pg)
            state["gtiles"][pg]["done"] = True
        phase(ops)
        phase(flush_at(gmax=99))

        nc.sync.dma_start(out=acc[:, :], in_=acc_sb)

    nc.finalize()
    return nc


def _get_nc():
    global _nc_cache
    if _nc_cache is None:
        _nc_cache = _build_nc()
    return _nc_cache


_sym_idx_cache = None


def _sym_idx():
    global _sym_idx_cache
    if _sym_idx_cache is None:
        p = np.arange(128)[:, None, None, None]
        c2 = np.arange(N_C2)[None, :, None, None]
        j = np.arange(2)[None, None, :, None]
        s = np.arange(S_PER_CORE)[None, None, None, :]
        _sym_idx_cache = (256 * c2 + 128 * j + p + s).astype(np.int64)
    return _sym_idx_cache


def kernel(vis_states: np.ndarray, weights: np.ndarray) -> np.ndarray:
    global last_results
    vis = np.asarray(vis_states).astype(np.float32)
    v = 2.0 * vis - 1.0                       # {-1, +1}
    vv = np.concatenate([v, v]).astype(ml_dtypes.float8_e4m3)  # exact in fp8
    w = np.asarray(weights)

    # quantize scaled weights to e4m3 (TRN FP8_EXP4 max +-240)
    wr = np.clip(w.real.astype(np.float64) * SCALE, -240, 240)
    wi = np.clip(w.imag.astype(np.float64) * SCALE, -240, 240)
    wr8 = wr.astype(ml_dtypes.float8_e4m3)
    wi8 = wi.astype(ml_dtypes.float8_e4m3)

    # wt[ob, r, p, c2, j, o] = W8[r][ob*128+o, (2*c2+j)*128+p]
    wt = np.empty((N_OB, 2, 128, N_C2, 2, 128), dtype=ml_dtypes.float8_e4m3)
    for r, w8 in ((0, wr8), (1, wi8)):
        a = w8.T.reshape(N_C2, 2, 128, N_OB, 128)   # [c2, j, p, ob, o]
        wt[:, r] = a.transpose(3, 2, 0, 1, 4)        # [ob, p, c2, j, o]

    idx = _sym_idx()
    in_maps = []
    for c in range(N_CORES):
        win = vv[c * S_PER_CORE : c * S_PER_CORE + WIN]
        in_maps.append({"symd": np.ascontiguousarray(win[idx]), "wt": wt})

    nc = _get_nc()
    res = run_bass_kernel_spmd(nc, in_maps, core_ids=list(range(N_CORES)))
    last_results = res

    tot_ln = 0.0
    tot_at = 0.0
    for r in res.results:
        a = r["acc"].astype(np.float64)
        tot_ln += a[:, 0::2].sum()
        tot_at += a[:, 1::2].sum()

    n_counted = N_CORES * S_PER_CORE * OUP_N  # includes the wrap shift s=4095
    real = 0.5 * tot_ln - math.log(2.0) * n_counted
    imag = 2.0 * tot_at

    # subtract the wrap shift's exact contribution (same quantized weights)
    w_eff = (wr8.astype(np.float64) + 1j * wi8.astype(np.float64)) / SCALE
    v4095 = v.astype(np.float64)[(4095 + np.arange(INP_N)) % VIS_N]
    pre = w_eff @ v4095
    f4095 = np.sum(np.log(np.cosh(pre)))
    real -= f4095.real
    imag -= f4095.imag
    return np.array(real + 1j * imag, dtype=np.complex64)
